# revision 55
# baseline (speedup 1.0000x reference)
"""Trainium2 Bass kernel for fused GQA attention block (B=2, L=2048, D=2048,
H=16 q-heads, KV=4 kv-heads, HD=64, causal, QK-RMSNorm + RoPE).

Sharding (8 cores): core c -> batch b = c // 4, head-group g = c % 4
(query heads 4g..4g+3, kv head g). Each core computes its 4 heads'
attention and a partial output projection (256 of 1024 e-channels);
host sums the 4 partials per batch.
"""

import os

import numpy as np

import concourse.bass as bass
import concourse.mybir as mybir
import concourse.tile as tile
from concourse import bacc
from concourse import bass_utils
from concourse.masks import make_identity

F32 = mybir.dt.float32
F32R = mybir.dt.float32r
BF16 = mybir.dt.bfloat16
AF = mybir.ActivationFunctionType
ALU = mybir.AluOpType

B, L, D = 2, 2048, 2048
H, KV, HD = 16, 4, 64
EPS = 1e-6
ROPE_BASE = 10000.0
N_CORES = 8
GQ = H // KV          # 4 query heads per core
LT = L // 128         # 16 l-tiles
DT = D // 128         # 16 d-tiles (contraction tiles for qkv proj)
TQ = 512              # q-chunk width for attention
NQC = L // TQ         # 4 q-chunks
NKB = L // 128        # 16 k-blocks
EW = (GQ + 2) * HD    # 384 qkv channels per core
EO = GQ * HD          # 256 output channels per core


def _classify_mask(mask):
    """Per (kb, qt) block: 'skip' | 'full' | pattern index into mixed list.

    Patterns are transposed slices maskT[k0:k0+128, q0:q0+TQ]."""
    kinds = {}
    patterns = []
    pat_ids = {}
    deltas = {}
    for qt in range(NQC):
        for kb in range(NKB):
            sub = mask[qt * TQ:(qt + 1) * TQ, kb * 128:(kb + 1) * 128]
            if np.all(sub <= -1e8):
                kinds[(kb, qt)] = "skip"
            elif np.all(sub == 0.0):
                kinds[(kb, qt)] = "full"
            else:
                pt = np.ascontiguousarray(sub.T.astype(np.float32))
                key = pt.tobytes()
                if key not in pat_ids:
                    pat_ids[key] = len(patterns)
                    patterns.append(pt)
                kinds[(kb, qt)] = pat_ids[key]
                # affine (causal-boundary) pattern? keep iff q >= k
                delta = kb * 128 - qt * TQ
                kk = np.arange(128)[:, None]
                qq = np.arange(TQ)[None, :]
                causal = np.where(qq >= kk + delta, 0.0, -1e9).astype(np.float32)
                deltas[(kb, qt)] = delta if np.array_equal(pt, causal) else None
    return kinds, patterns, deltas


KOPT_ZACT = os.environ.get("KOPT_ZACT", "1") == "1"
KOPT_TTR = os.environ.get("KOPT_TTR", "0") == "1"
KOPT_XSYNC = os.environ.get("KOPT_XSYNC", "1") == "1"
KOPT_PHASES = os.environ.get("KOPT_PHASES", "123")
KOPT_P2 = os.environ.get("KOPT_P2", "egn")
KOPT_RECIP = os.environ.get("KOPT_RECIP", "exact")
# new knobs (defaults = optimized)
KOPT_TRIM = os.environ.get("KOPT_TRIM", "1") == "1"      # causal col trim
KOPT_AVB = int(os.environ.get("KOPT_AVB", "1"))          # av psum bufs
KOPT_SCB = int(os.environ.get("KOPT_SCB", "2"))          # scores psum bufs
KOPT_ZOENG = os.environ.get("KOPT_ZOENG", "dada")        # zo copy engines
KOPT_RSTD = os.environ.get("KOPT_RSTD", "sqrt")          # rstd path
KOPT_STATS = os.environ.get("KOPT_STATS", "dve2")        # sumsq engine
KOPT_TPR = os.environ.get("KOPT_TPR", "1") == "1"        # f32r transposes
KOPT_TCPY = os.environ.get("KOPT_TCPY", "dddd")          # qp_lo/qp_hi/kt0/kt1
KOPT_WFOLD = os.environ.get("KOPT_WFOLD", "1") == "1"    # const-w -> exp scale
KOPT_ROPE = os.environ.get("KOPT_ROPE", "pppddd")        # rope op engines
KOPT_QN = os.environ.get("KOPT_QN", "d")                 # qn mult engine
KOPT_BF16 = os.environ.get("KOPT_BF16", "0") == "1"      # bf16 q/k/v path
KOPT_TPK = os.environ.get("KOPT_TPK", "av")              # tpk psum pool
KOPT_TPKB = int(os.environ.get("KOPT_TPKB", "2"))        # tpk pool bufs


def _build_program(kinds, n_mixed, repeat=1, deltas=None, W_FOLDED=False,
                   exp_scale=None):
    # W_FOLDED doubles as the exp_scale carrier (float) so the host-prep
    # return stays a 5-tuple for test.py
    if isinstance(W_FOLDED, float):
        exp_scale = W_FOLDED
        W_FOLDED = False
    nc = bacc.Bacc("TRN2", target_bir_lowering=False, debug=False,
                   enable_asserts=False, num_devices=N_CORES)

    use_bf16 = KOPT_BF16 and exp_scale is not None
    XWDT = BF16 if use_bf16 else F32R     # x, W_qkv operands
    QDT = BF16 if use_bf16 else F32       # q5 / rope scratch
    RDT = BF16 if use_bf16 else (F32R if KOPT_TPR else F32)   # rq + transposes
    PDT = BF16 if use_bf16 else F32R      # qpair/kt/vt/p

    # DRAM I/O (per core). Host pre-tiles everything into DMA-friendly
    # layouts. x3[lt, p, t, j] = x[lt*128+j, t*128+p]: per-partition data is
    # fully contiguous (8KB descriptors instead of 512B).
    xT = nc.dram_tensor("xT", [LT, 128, DT * 128], XWDT,
                        kind="ExternalInput").ap()
    wqkT = nc.dram_tensor("wqkT", [D, EW], XWDT, kind="ExternalInput").ap()
    woT = nc.dram_tensor("woT", [EO, D], F32R, kind="ExternalInput").ap()
    cos2 = nc.dram_tensor("cos2", [128, LT * 32], F32, kind="ExternalInput").ap()
    sin2 = nc.dram_tensor("sin2", [128, LT * 32], F32, kind="ExternalInput").ap()
    qw = nc.dram_tensor("qw", [128, GQ * HD], F32, kind="ExternalInput").ap()
    kw = nc.dram_tensor("kw", [128, HD], F32, kind="ExternalInput").ap()
    if n_mixed:
        mblk = nc.dram_tensor("mblk", [128, n_mixed * TQ], F32R,
                              kind="ExternalInput").ap()
    y = nc.dram_tensor("y", [L, D], F32, kind="ExternalOutput").ap()

    with tile.TileContext(nc) as tc:
        with (
            tc.tile_pool(name="consts", bufs=1) as consts,
            tc.tile_pool(name="wpool", bufs=1) as wpool,
            tc.tile_pool(name="xcolp", bufs=3) as xcolp,
            tc.tile_pool(name="work", bufs=4) as work,
            tc.tile_pool(name="persist", bufs=1) as persist,
            tc.tile_pool(name="pp", bufs=4) as pp,
            tc.tile_pool(name="zp", bufs=4) as zp,
            tc.tile_pool(name="ps_a", bufs=2, space="PSUM") as ps_a,
            tc.tile_pool(name="ps_b", bufs=KOPT_AVB, space="PSUM") as ps_b,
            tc.tile_pool(name="ps_sc", bufs=KOPT_SCB, space="PSUM") as ps_sc,
            tc.tile_pool(name="ps_tpk", bufs=KOPT_TPKB, space="PSUM")
            as ps_tpk,
        ):
            # ---- constants ----
            ident = consts.tile([128, 128], F32, tag="ident")
            make_identity(nc, ident[:])
            cos_sb = consts.tile([128, LT * 32], F32, tag="cos")
            sin_sb = consts.tile([128, LT * 32], F32, tag="sin")
            nc.sync.dma_start(cos_sb[:], cos2[:])
            nc.sync.dma_start(sin_sb[:], sin2[:])
            eps_sb = consts.tile([128, 1], F32, tag="eps")
            nc.vector.memset(eps_sb[:], EPS)
            # w5 = [qw x4 | kw] (scale folded into qw on host)
            w5_sb = consts.tile([128, (GQ + 1) * HD], F32, tag="w5")
            nc.sync.dma_start(w5_sb[:, 0:GQ * HD], qw[:])
            nc.sync.dma_start(w5_sb[:, GQ * HD:(GQ + 1) * HD], kw[:])
            if n_mixed:
                # fp32r for PE-side mask accumulate
                mb_sb = consts.tile([128, n_mixed * TQ], F32R, tag="mb")
                nc.sync.dma_start(mb_sb[:], mblk[:])
            ident_r = consts.tile([128, 128], F32R, tag="ident_r")
            nc.vector.tensor_copy(ident_r[:], ident[:])
            if use_bf16:
                ident_t = consts.tile([128, 128], BF16, tag="ident_t")
                nc.vector.tensor_copy(ident_t[:], ident[:])
            else:
                ident_t = ident_r if KOPT_TPR else ident

            # ---- weights (rounded during SWDGE cast-DMA / host cast) ----
            wqk_sb = []
            for dt_i in range(DT):
                w = wpool.tile([128, EW], XWDT, tag=f"wqk{dt_i}")
                nc.sync.dma_start(w[:], wqkT[dt_i * 128:(dt_i + 1) * 128, :])
                wqk_sb.append(w)
            wo_sb = []
            for et in range(2):
                w = wpool.tile([128, D], F32R, tag=f"wo{et}")
                nc.sync.dma_start(w[:], woT[et * 128:(et + 1) * 128, :])
                wo_sb.append(w)

            # ---- persistent attention operands ----
            # Q^T head pairs stacked on partitions: qpair[i] rows 0-63 = head 2i,
            # rows 64-127 = head 2i+1. K^T duplicated on both halves (shared kv).
            # V-hat [128 = V|ones, 128 per k-tile].
            qpair2 = persist.tile([128, (GQ // 2) * L], PDT, tag="qpair2")
            kt_sb = persist.tile([128, L], PDT, tag="kt")
            vt_sb = persist.tile([128, LT * 128], PDT, tag="vt")
            # ones block: cols HD..127 of each k-tile group (rows of denom)
            ones_sb = consts.tile([128, HD], PDT if use_bf16 else F32,
                                  tag="ones")
            nc.vector.memset(ones_sb[:], 1.0)
            for i in range(LT):
                nc.vector.tensor_copy(
                    vt_sb[:, i * 128 + HD:(i + 1) * 128], ones_sb[:])
            aot_sb = [persist.tile([128, L], F32R, tag=f"aot{et}",
                                   name=f"aot{et}")
                      for et in range(2)]

            def emit_transposes(lt, rq):
                # transposes of l-tile lt (PSUM base 0 required); pair-stacking
                # happens in the DVE partition-shift copies below.
                tdt = RDT
                if use_bf16:
                    # bf16: Q (512 cols) + K (128 cols) transposes pack into
                    # one PSUM bank; av pool stays free for early phase-2 AV
                    tp = ps_a.tile([64, 640], tdt, tag="mm_a")
                else:
                    tp = ps_a.tile([64, 512], tdt, tag="mm_a")
                tident = ident_t
                for h in range(GQ):
                    nc.tensor.matmul(
                        tp[:, h * 128:(h + 1) * 128],
                        rq[:, h * HD:(h + 1) * HD],
                        tident[:], is_transpose=True,
                        skip_group_check=True)
                # k transpose: packed next to Q (bf16), own pool ('sep'), or
                # borrowing the av slot ('av' — serializes all phase-2 AV
                # matmuls behind phase 1!)
                if use_bf16:
                    tpk = tp[:, 512:640]
                elif KOPT_TPK == "sep":
                    tpk_t = ps_tpk.tile([64, 128], tdt, tag="tpk",
                                        name="tpk_t")
                    tpk = tpk_t[:]
                else:
                    tpk_t = ps_b.tile([64, 128], tdt, tag="av", name="tpk_t")
                    tpk = tpk_t[:]
                nc.tensor.matmul(tpk, rq[:, GQ * HD:(GQ + 1) * HD],
                                 tident[:], is_transpose=True)
                tpv = tp[:, 0:512].rearrange("p (r c) -> p r c", c=256)
                qp_lo = qpair2[0:64, :].rearrange(
                    "p (r j) -> p r j", j=L)[:, :, lt * 128:(lt + 1) * 128]
                qp_hi = qpair2[64:128, :].rearrange(
                    "p (r j) -> p r j", j=L)[:, :, lt * 128:(lt + 1) * 128]
                def pcpy(which, dst, src):
                    # PSUM source: only DVE ('d') or ACT ('a') have a port
                    if KOPT_TCPY[which] == "a":
                        nc.scalar.copy(dst, src)
                    else:
                        nc.vector.tensor_copy(dst, src)
                pcpy(0, qp_lo, tpv[:, :, 0:128])
                pcpy(1, qp_hi, tpv[:, :, 128:256])
                pcpy(2, kt_sb[0:64, lt * 128:(lt + 1) * 128], tpk)
                pcpy(3, kt_sb[64:128, lt * 128:(lt + 1) * 128], tpk)

            def emit_body():
                # ================= Phase 1: QKV + RMSNorm + RoPE =================
                # software-pipelined: transposes of tile lt-1 are emitted after
                # the QKV matmuls of tile lt so PE never waits on DVE rope.
                prev = [None]

                def emit_p1(lt):
                    xcol = xcolp.tile([128, D], XWDT, tag="xcol")
                    xeng = nc.sync if KOPT_XSYNC else nc.gpsimd
                    xeng.dma_start(xcol[:], xT[lt, :, :])
                    qkv_ps = ps_a.tile([128, EW], F32, tag="mm_a")
                    for dt_i in range(DT):
                        nc.tensor.matmul(
                            qkv_ps[:], xcol[:, dt_i * 128:(dt_i + 1) * 128],
                            wqk_sb[dt_i][:],
                            start=(dt_i == 0), stop=(dt_i == DT - 1))
                    if prev[0] is not None:
                        emit_transposes(prev[0][0], prev[0][1])

                    # RMS stats for 5 norm groups (4 q heads + 1 k head).
                    # q5 stages qkv out of PSUM so the mm_a slot frees early.
                    q5 = work.tile([128, (GQ + 1) * HD], QDT, tag="q5")
                    nc.vector.tensor_copy(q5[:], qkv_ps[:, 0:(GQ + 1) * HD])
                    # V copy right away so the qkv PSUM slot frees early
                    nc.vector.tensor_copy(
                        vt_sb[:, lt * 128:lt * 128 + HD],
                        qkv_ps[:, (GQ + 1) * HD:(GQ + 2) * HD])
                    sq_scr = work.tile([128, HD], QDT, tag="sq_scr")
                    ss = work.tile([128, 16], F32, tag="ss")
                    if KOPT_STATS == "dve2":
                        # native DVE ops (tensor_tensor_reduce ucode crashes
                        # the device): square all 5 groups, then X-reduce
                        sq5 = work.tile([128, (GQ + 1) * HD], BF16, tag="sq5")
                        nc.vector.tensor_tensor(sq5[:], q5[:], q5[:],
                                                op=ALU.mult)
                        nc.vector.tensor_reduce(
                            ss[:, 0:GQ + 1],
                            sq5[:].rearrange("p (h e) -> p h e", e=HD),
                            mybir.AxisListType.X, ALU.add)
                    elif KOPT_STATS == "dve":
                        for i in range(GQ + 1):
                            nc.vector.tensor_tensor_reduce(
                                sq_scr[:], q5[:, i * HD:(i + 1) * HD],
                                q5[:, i * HD:(i + 1) * HD], 1.0, 0.0,
                                ALU.mult, ALU.add, ss[:, i:i + 1])
                    else:
                        for i in range(GQ + 1):
                            nc.scalar.activation(
                                sq_scr[:], q5[:, i * HD:(i + 1) * HD],
                                AF.Square, accum_out=ss[:, i:i + 1])
                    rstd = work.tile([128, GQ + 1], F32, tag="rstd")
                    if KOPT_RSTD == "lnexp":
                        # rstd = exp(-0.5*ln(ss/HD + eps)); ln+exp share the
                        # exp activation table (sqrt does not)
                        nc.scalar.activation(ss[:, 5:5 + GQ + 1],
                                             ss[:, 0:GQ + 1],
                                             AF.Ln, bias=eps_sb[:],
                                             scale=1.0 / HD)
                        nc.scalar.activation(rstd[:], ss[:, 5:5 + GQ + 1],
                                             AF.Exp, scale=-0.5)
                    else:
                        # rstd = 1/sqrt(ss/HD + eps)
                        nc.scalar.activation(ss[:, 5:5 + GQ + 1],
                                             ss[:, 0:GQ + 1],
                                             AF.Sqrt, bias=eps_sb[:],
                                             scale=1.0 / HD)
                        nc.vector.reciprocal(rstd[:], ss[:, 5:5 + GQ + 1])

                    G5 = GQ + 1
                    cs = cos_sb[:, lt * 32:(lt + 1) * 32]
                    sn = sin_sb[:, lt * 32:(lt + 1) * 32]
                    csq = cs[:, None, :].broadcast_to([128, G5, 32])
                    snq = sn[:, None, :].broadcast_to([128, G5, 32])
                    rq = work.tile([128, G5 * HD], RDT, tag="rq")
                    rqv = rq[:].rearrange("p (h e) -> p h e", e=HD)
                    t1 = work.tile([128, G5 * 32], QDT, tag="t1")
                    t1v = t1[:].rearrange("p (h e) -> p h e", e=32)
                    t2 = work.tile([128, G5 * 32], QDT, tag="t2")
                    t2v = t2[:].rearrange("p (h e) -> p h e", e=32)

                    def reng(i):
                        return nc.gpsimd if KOPT_ROPE[i] == "p" else nc.vector

                    if exp_scale is not None:
                        # rstd-scaled cos/sin per head (rope is linear, so
                        # rope(q*rstd) = rope via scaled tables); materialized
                        # so the rope ops below have no broadcast operands
                        csr = work.tile([128, G5 * 32], QDT, tag="csr")
                        csrv = csr[:].rearrange("p (h e) -> p h e", e=32)
                        snr = work.tile([128, G5 * 32], QDT, tag="snr")
                        snrv = snr[:].rearrange("p (h e) -> p h e", e=32)
                        rsb = rstd[:, 0:G5, None].broadcast_to([128, G5, 32])
                        nc.vector.tensor_tensor(csrv, csq, rsb, op=ALU.mult)
                        nc.vector.tensor_tensor(snrv, snq, rsb, op=ALU.mult)
                        qnv = q5[:].rearrange("p (h e) -> p h e", e=HD)
                    else:
                        # general path: explicit normalize * weight, plain
                        # cos/sin tables (rope ops stay on DVE: broadcasts)
                        qn = work.tile([128, G5 * HD], F32, tag="qn")
                        nc.vector.tensor_tensor(
                            qn[:].rearrange("p (h e) -> p h e", e=HD),
                            q5[:].rearrange("p (h e) -> p h e", e=HD),
                            rstd[:, 0:G5, None].broadcast_to([128, G5, HD]),
                            op=ALU.mult)
                        if not W_FOLDED:
                            nc.vector.tensor_tensor(qn[:], qn[:], w5_sb[:],
                                                    op=ALU.mult)
                        csrv, snrv = csq, snq
                        qnv = qn[:].rearrange("p (h e) -> p h e", e=HD)

                        def reng(i):  # noqa: F811
                            return nc.vector

                    # low half: x1*cos - x2*sin   (separate t1 scratch)
                    reng(0).tensor_tensor(t1v, qnv[:, :, 0:32], csrv,
                                          op=ALU.mult)
                    reng(1).tensor_tensor(rqv[:, :, 0:32], qnv[:, :, 32:64],
                                          snrv, op=ALU.mult)
                    reng(2).tensor_tensor(rqv[:, :, 0:32], t1v,
                                          rqv[:, :, 0:32], op=ALU.subtract)
                    # high half: x1*sin + x2*cos  (t2 scratch)
                    reng(3).tensor_tensor(t2v, qnv[:, :, 0:32], snrv,
                                          op=ALU.mult)
                    reng(4).tensor_tensor(rqv[:, :, 32:64], qnv[:, :, 32:64],
                                          csrv, op=ALU.mult)
                    reng(5).tensor_tensor(rqv[:, :, 32:64], t2v,
                                          rqv[:, :, 32:64], op=ALU.add)

                    prev[0] = (lt, rq)

                # ================= Phase 2: attention =================
                # head pairs run as concurrent row-tiled (K=64) matmuls
                def emit_p2(qc):
                    klist = [kb for kb in range(NKB)
                             if kinds[(kb, qc)] != "skip"]
                    if not klist:
                        return
                    for pr in range(GQ // 2):
                        av_ps = ps_b.tile([128, 2 * TQ], F32, tag="av")
                        first = True
                        qsl = qpair2[:, pr * L + qc * TQ:pr * L + (qc + 1) * TQ]
                        for ci, kb in enumerate(klist):
                            kind = kinds[(kb, qc)]
                            # causal column trim: q-cols < delta are fully
                            # masked; skip them in scores/exp/AV. Keep MM
                            # free-dim >= 256 (fp32r slows below that).
                            delta = None if kind == "full" else deltas[(kb, qc)]
                            trim = 0
                            if (KOPT_TRIM and delta is not None
                                    and "g" in KOPT_P2):
                                trim = max(0, min(delta, TQ - 256))
                            w = TQ - trim
                            sc_ps = ps_sc.tile([128, 1024], F32, tag="sc")
                            for sub in range(2):
                                nc.tensor.matmul(
                                    sc_ps[:, sub * TQ + trim:(sub + 1) * TQ],
                                    kt_sb[sub * 64:(sub + 1) * 64,
                                          kb * 128:(kb + 1) * 128],
                                    qsl[sub * 64:(sub + 1) * 64, trim:TQ],
                                    start=True,
                                    stop=(kind == "full"
                                          or "m" not in KOPT_P2))
                                if kind != "full" and "m" in KOPT_P2:
                                    # additive mask via PE accumulate
                                    nc.tensor.matmul(
                                        sc_ps[:, sub * TQ:(sub + 1) * TQ],
                                        ident_r[:],
                                        mb_sb[:, kind * TQ:(kind + 1) * TQ],
                                        start=False, stop=True)
                                elif kind != "full" and "d" in KOPT_P2:
                                    nc.vector.tensor_tensor(
                                        sc_ps[:, sub * TQ:(sub + 1) * TQ],
                                        sc_ps[:, sub * TQ:(sub + 1) * TQ],
                                        mb_sb[:, kind * TQ:(kind + 1) * TQ]
                                        .bitcast(F32),
                                        op=ALU.add)
                            p_sb = pp.tile([128, 1024], PDT, tag="p")
                            if trim:
                                pv3 = p_sb[:].rearrange(
                                    "p (s q) -> p s q", q=TQ)[:, :, trim:TQ]
                                sv3 = sc_ps[:].rearrange(
                                    "p (s q) -> p s q", q=TQ)[:, :, trim:TQ]
                            else:
                                pv3, sv3 = p_sb[:], sc_ps[:]
                            if "e" in KOPT_P2:
                                nc.scalar.activation(
                                    pv3, sv3, AF.Exp,
                                    scale=(1.0 if exp_scale is None
                                           else float(exp_scale)))
                            else:
                                nc.vector.tensor_copy(pv3, sv3)
                            if kind != "full" and "g" in KOPT_P2:
                                # causal boundary: zero entries with q < k
                                # on the (otherwise idle) GPSIMD engine
                                assert delta is not None
                                qa1 = min(delta + 128, TQ)
                                wa = qa1 - trim
                                pva = p_sb[:].rearrange(
                                    "p (s q) -> p s q", q=TQ)[:, :, trim:qa1]
                                nc.gpsimd.affine_select(
                                    pva, pva,
                                    pattern=[[0, 2], [1, wa]],
                                    compare_op=ALU.is_ge,
                                    fill=0.0,
                                    base=trim - delta,
                                    channel_multiplier=-1)
                            last = (ci == len(klist) - 1)
                            for sub in range(2):
                                nc.tensor.matmul(
                                    av_ps[:, sub * TQ + trim:(sub + 1) * TQ],
                                    vt_sb[:, kb * 128:(kb + 1) * 128],
                                    p_sb[:, sub * TQ + trim:(sub + 1) * TQ],
                                    start=first, stop=last)
                            first = False
                        # normalize: rows HD..2*HD hold the denominator
                        for sub in range(2):
                            avs = av_ps[:, sub * TQ:(sub + 1) * TQ]
                            aslice = aot_sb[pr][sub * 64:(sub + 1) * 64,
                                                qc * TQ:(qc + 1) * TQ]
                            if "v" in KOPT_P2:
                                dcp = work.tile([HD, TQ], F32, tag="rec")
                                nc.vector.tensor_copy(dcp[:], avs[HD:2 * HD, :])
                                nc.vector.tensor_tensor(
                                    aslice, avs[0:HD, :], dcp[:],
                                    op=ALU.divide)
                            elif "n" in KOPT_P2:
                                rec = work.tile([HD, TQ], F32, tag="rec")
                                if KOPT_RECIP == "fast":
                                    nc.vector.reciprocal_approx_fast(
                                        rec[:], avs[HD:2 * HD, :])
                                elif KOPT_RECIP == "approx":
                                    rscr = work.tile([HD, TQ], F32, tag="rscr")
                                    nc.vector.reciprocal_approx_accurate(
                                        rec[:], avs[HD:2 * HD, :], rscr[:])
                                else:
                                    nc.vector.reciprocal(rec[:],
                                                         avs[HD:2 * HD, :])
                                nc.vector.tensor_tensor(
                                    aslice, avs[0:HD, :], rec[:], op=ALU.mult)
                            else:
                                nc.vector.tensor_copy(aslice, avs[0:HD, :])

                # ================= Phase 3: output projection =================
                def emit_p3(lt):
                    zo = zp.tile([128, 2048], F32, tag="zo")
                    for dc in range(4):
                        z_ps = ps_a.tile([128, 512], F32, tag="mm_a")
                        for et in range(2):
                            nc.tensor.matmul(
                                z_ps[:], aot_sb[et][:, lt * 128:(lt + 1) * 128],
                                wo_sb[et][:, dc * 512:(dc + 1) * 512],
                                start=(et == 0), stop=(et == 1))
                        zslice = zo[:, dc * 512:(dc + 1) * 512]
                        zeng = KOPT_ZOENG[dc % len(KOPT_ZOENG)]
                        if zeng == "a" and KOPT_ZACT:
                            nc.scalar.copy(zslice, z_ps[:])
                        else:
                            nc.vector.tensor_copy(zslice, z_ps[:])
                    nc.sync.dma_start(y[lt * 128:(lt + 1) * 128, :], zo[:])

                # driver: sequential phases (interleaving measured slower --
                # PSUM slot and engine-order contention outweigh the overlap)
                if "1" in KOPT_PHASES:
                    for lt in range(LT):
                        emit_p1(lt)
                    if prev[0] is not None:
                        emit_transposes(prev[0][0], prev[0][1])
                        prev[0] = None
                if "2" in KOPT_PHASES:
                    for qc in range(NQC):
                        emit_p2(qc)
                if "3" in KOPT_PHASES:
                    for lt in range(LT):
                        emit_p3(lt)

            if repeat > 1:
                with tc.For_i(0, repeat, 1):
                    emit_body()
            else:
                emit_body()

    nc.compile()
    return nc


_PROGRAM_CACHE = {}


def _get_program(kinds, n_mixed, repeat=1, deltas=None, W_FOLDED=False,
                 exp_scale=None):
    key = (tuple(sorted(kinds.items())), n_mixed, repeat, W_FOLDED, exp_scale)
    if key not in _PROGRAM_CACHE:
        _PROGRAM_CACHE[key] = _build_program(kinds, n_mixed, repeat, deltas,
                                             W_FOLDED, exp_scale)
    return _PROGRAM_CACHE[key]


def _host_prep(x, W_qkv, W_out, q_norm_w, k_norm_w, mask):
    kinds, patterns, deltas = _classify_mask(np.asarray(mask))
    n_mixed = len(patterns)
    assert n_mixed <= 12, f"too many unique mask patterns: {n_mixed}"

    # RoPE tables, tiled [128, LT*32]: cos2[p, lt*32+j] = cos((lt*128+p)*freq_j)
    j = np.arange(0, HD, 2, dtype=np.float32)
    freqs = (ROPE_BASE ** (-j / HD)).astype(np.float32)
    pos = np.arange(L, dtype=np.float32)
    theta = pos[:, None] * freqs[None, :]
    cosf = np.cos(theta).astype(np.float32)     # [L, 32]
    sinf = np.sin(theta).astype(np.float32)
    cos2 = np.ascontiguousarray(
        cosf.reshape(LT, 128, 32).transpose(1, 0, 2).reshape(128, LT * 32))
    sin2 = np.ascontiguousarray(
        sinf.reshape(LT, 128, 32).transpose(1, 0, 2).reshape(128, LT * 32))

    scale = np.float32(HD ** -0.5)
    qwv = (np.asarray(q_norm_w, np.float32) * scale)
    kwv = np.asarray(k_norm_w, np.float32)
    # constant norm weights commute with rope+rms: fold the whole constant
    # (w_q * w_k * HD^-0.5) into the softmax exp's pre-scale instead of
    # multiplying on-chip.
    qn_ = np.asarray(q_norm_w, np.float32)
    kn_ = np.asarray(k_norm_w, np.float32)
    exp_scale = None
    if (KOPT_WFOLD and np.all(qn_ == qn_[0]) and np.all(kn_ == kn_[0])):
        exp_scale = float(qn_[0]) * float(kn_[0]) * float(scale)
    # constant norm weights commute with RMS (up to a negligible eps shift):
    # fold them into W_qkv on the host and skip the on-chip multiply
    w_folded = exp_scale if exp_scale is not None else False
    qw_rep = np.tile(np.tile(qwv, GQ)[None, :], (128, 1)).astype(np.float32)
    kw_rep = np.tile(kwv[None, :], (128, 1))

    if n_mixed:
        mb = np.concatenate(patterns, axis=1).astype(np.float32)  # [128, nm*TQ]
    else:
        mb = None

    use_bf16 = KOPT_BF16 and exp_scale is not None
    np_bf16 = mybir.dt.np(BF16)
    in_maps = []
    xTt_by_batch = {}
    for c in range(N_CORES):
        b, g = divmod(c, KV)
        if b not in xTt_by_batch:
            xb = np.asarray(x[b], np.float32)
            # x3[lt, p, t, j] = x[lt*128+j, t*128+p]; contiguous per (lt, p)
            xTt_b = np.ascontiguousarray(
                xb.reshape(LT, 128, DT, 128).transpose(0, 3, 2, 1)
                .reshape(LT, 128, DT * 128))
            if use_bf16:
                xTt_b = xTt_b.astype(np_bf16)
            xTt_by_batch[b] = xTt_b
        xTt = xTt_by_batch[b]
        rows = np.r_[g * GQ * HD:(g + 1) * GQ * HD,
                     (H + g) * HD:(H + g + 1) * HD,
                     (H + KV + g) * HD:(H + KV + g + 1) * HD]
        wqkT = np.ascontiguousarray(np.asarray(W_qkv, np.float32)[rows].T)
        if w_folded is True:
            wqkT[:, 0:GQ * HD] *= qwv[0]
            wqkT[:, GQ * HD:(GQ + 1) * HD] *= kwv[0]
        if use_bf16:
            wqkT = wqkT.astype(np_bf16)
        cols = np.arange(g * GQ * HD, (g + 1) * GQ * HD)
        woT = np.ascontiguousarray(np.asarray(W_out, np.float32)[:, cols].T)
        m = {"xT": xTt, "wqkT": wqkT, "woT": woT,
             "cos2": cos2, "sin2": sin2, "qw": qw_rep, "kw": kw_rep}
        if mb is not None:
            m["mblk"] = mb
        in_maps.append(m)
    return kinds, n_mixed, in_maps, deltas, w_folded


def kernel(x, W_qkv, W_out, q_norm_w, k_norm_w, mask):
    kinds, n_mixed, in_maps, deltas, wf = _host_prep(x, W_qkv, W_out,
                                                     q_norm_w, k_norm_w, mask)
    nc = _get_program(kinds, n_mixed, deltas=deltas, W_FOLDED=wf)
    res = bass_utils.run_bass_kernel_spmd(nc, in_maps,
                                          core_ids=list(range(N_CORES)))
    out = np.zeros((B, L, D), dtype=np.float32)
    for c in range(N_CORES):
        b = c // KV
        out[b] += res.results[c]["y"]
    return out

